# revision 28
# baseline (speedup 1.0000x reference)
"""Multi-head attention (B=8, C=512, L=2048, H=8, D=64) on 8 TRN2 NeuronCores.

Sharding: pure batch-parallel - core b computes batch b end-to-end (qkv proj,
8 heads of attention, out proj). No collectives.

Per-core layout strategy:
  - qkv projection with lhsT = w_qkv.T (host-transposed), rhs = x.
  - S^T = K^T Q  (keys on partitions) so the exp output is already the
    transposed P^T needed by the PV matmul, and no max-subtraction is needed
    (scores are ~N(0,1) after the 1/sqrt(D) scale, folded into exp's scale).
  - Heads are processed in pairs (2t, 2t+1) that live in partition halves
    0-63 / 64-127 of one qkv row-tile. The two K=64 S^T matmuls of a pair
    run CONCURRENTLY in the PE array (row groups 0-1 vs 2-3) and write the
    two 512-column halves of one [128, 1024] PSUM tile, so a single
    ScalarE exp instruction covers both heads.
  - PV uses lhsT = [V^T | ones] (65 columns): row 64 of the accumulator is
    the softmax denominator, computed for free.
  - V^T is computed directly from X (lhsT = X tiles), V is never materialized.
  - i is processed in 512-wide chunks (outer loop) so each chunk of the
    output projection overlaps the next chunk's attention pass.
"""

import os
import sys

sys.path.insert(0, "/opt/trn_rl_repo")

import numpy as np
import ml_dtypes

import concourse.bass as bass
import concourse.tile as tile
from concourse import bacc, mybir
from concourse import bass_utils

# ---- custom DVE exp: p = poly4(v), then p^16 (v = 0.125*S/16) -------------
# Offloads part of the softmax exp from the (bottleneck) ScalarE to VectorE.
from concourse.dve_spec import Spec, Src0, C0, C1, C2, One, sq, lower, _has_src1
import concourse.dve_ops as dve_ops
from concourse.dve_ops import DveOp
from concourse.dve_uop import DveOpSpec

EXP_C = (0.50053141, 0.16821747, 0.03882078)  # minimax on v in [-0.5125, 0.5125]


def _register_dve_op(name, spec):
    if name in dve_ops._SUB_OPCODE_FOR_NAME:
        return next(op for op in dve_ops.OPS if op.name == name)
    row = max(dve_ops._SUB_OPCODE_FOR_NAME.values()) + 1
    assert row < 0x20
    dve_ops._SUB_OPCODE_FOR_NAME[name] = row
    shas = {}
    for ver in ("v3", "v4"):
        s = DveOpSpec(
            name=name, opcode=row, uops=lower(spec, ver=ver), rd1_en=_has_src1(spec)
        )
        shas[ver] = s.sha(ver)
    op = DveOp(name, spec, subdim=False, uops_sha=shas)
    dve_ops.OPS.append(op)
    dve_ops.CUSTOM_DVE_SPECS[name] = spec
    return op


def _make_exp_ops():
    t = sq(Src0)
    spec1 = Spec(
        body=(One + Src0) + t * (C0 + C1 * Src0 + C2 * t),
        reference=lambda in0, in1, s0, s1, imm2: (
            1.0 + in0 + in0 * in0 * (s0 + s1 * in0 + imm2 * in0 * in0)
        ).astype(np.float32),
    )
    spec2 = Spec(
        body=sq(sq(sq(sq(Src0)))),
        reference=lambda in0, in1, s0, s1, imm2: (in0**16).astype(np.float32),
    )
    return (
        _register_dve_op("EXP16_POLY_ANT", spec1),
        _register_dve_op("POW16_ANT", spec2),
    )


EXP16_POLY, POW16 = _make_exp_ops()

# j-tiles (per 16-tile loop) whose exp runs on VectorE instead of ScalarE
DVE_JTS = frozenset(
    int(x) for x in os.environ.get("KERNEL_DVE_JTS", "4,9").split(",") if x != ""
)

B, C, L = 8, 512, 2048
H, D = 8, 64
HID = H * D  # 512
SCALE = float(D) ** -0.5
BF16 = mybir.dt.bfloat16
F32 = mybir.dt.float32
AF = mybir.ActivationFunctionType
NCORES = 8

NT = C // 128  # 4 channel tiles
NL = L // 512  # 4 l-chunks of 512
NJ = L // 128  # 16 key tiles


def build_kernel(tc, out_d, x_d, wqkvT_d, woutT_d, bias_d):
    nc = tc.nc
    from contextlib import ExitStack

    ctx = ExitStack()
    pers = ctx.enter_context(tc.tile_pool(name="pers", bufs=1))
    ptp = ctx.enter_context(tc.tile_pool(name="ptp", bufs=10))
    scrp = ctx.enter_context(tc.tile_pool(name="scrp", bufs=3))
    ytp = ctx.enter_context(tc.tile_pool(name="ytp", bufs=3))
    smp = ctx.enter_context(tc.tile_pool(name="smp", bufs=3))
    stp = ctx.enter_context(tc.tile_pool(name="stp", bufs=2, space="PSUM"))
    otp = ctx.enter_context(tc.tile_pool(name="otp", bufs=1, space="PSUM"))
    qkp = ctx.enter_context(tc.tile_pool(name="qkp", bufs=2, space="PSUM"))

    # ---- persistent SBUF tensors ----
    x_sb = [pers.tile([128, L], BF16, tag=f"x{c}", name=f"x{c}") for c in range(NT)]
    wq_sb = [
        pers.tile([128, 3 * HID], BF16, tag=f"wq{c}", name=f"wq{c}") for c in range(NT)
    ]
    wo_sb = [pers.tile([128, C], BF16, tag=f"wo{c}", name=f"wo{c}") for c in range(NT)]
    bias_sb = [
        pers.tile([128, 1], F32, tag=f"bias{c}", name=f"bias{c}") for c in range(NT)
    ]
    q_sb = [pers.tile([128, L], BF16, tag=f"q{t}", name=f"q{t}") for t in range(NT)]
    k_sb = [pers.tile([128, L], BF16, tag=f"k{t}", name=f"k{t}") for t in range(NT)]
    vt1 = [
        pers.tile([128, H * 65], BF16, tag=f"vt{j}", name=f"vt{j}") for j in range(NJ)
    ]
    o2 = [pers.tile([128, L], BF16, tag=f"o2_{c}", name=f"o2_{c}") for c in range(NT)]

    # ---- input DMAs (x first - it gates the qkv projection; halves engage
    # more DMA queues in parallel; wo/bias load later, off the hot path) ----
    for c in range(NT):
        r = slice(128 * c, 128 * (c + 1))
        nc.sync.dma_start(x_sb[c][:, 0:1024], x_d[r, 0:1024])
        nc.sync.dma_start(x_sb[c][:, 1024:2048], x_d[r, 1024:2048])
        nc.sync.dma_start(wq_sb[c][:, 0:1024], wqkvT_d[r, 0:1024])
    for c in range(NT):
        r = slice(128 * c, 128 * (c + 1))
        nc.sync.dma_start(wq_sb[c][:, 1024:1536], wqkvT_d[r, 1024:1536])

    # ---- PE warm-up: dummy matmuls during the input-DMA window so the HAM
    # clock gate opens (1.2 -> 2.4 GHz) before the real work arrives. The
    # chain ends in a DMA to an internal DRAM scratch so DCE keeps it. ----
    warm_scratch = nc.dram_tensor("warm_scratch", [128, 512], F32)
    warm_sb = pers.tile([128, 512], BF16, tag="warm", name="warm_sb")
    warm_out = pers.tile([128, 512], F32, tag="warmo", name="warm_out")
    nc.vector.memset(warm_sb[:, :], 0.001)
    wps = qkp.tile([128, 512], F32, tag="qkp", name="warm_ps")
    for w in range(40):
        nc.tensor.matmul(
            wps[:, :], lhsT=warm_sb[:, 0:128], rhs=warm_sb[:, :],
            start=True, stop=True,
        )
    nc.vector.tensor_copy(warm_out[:, :], wps[:, :])
    nc.sync.dma_start(warm_scratch.ap()[:, :], warm_out[:, :])

    def emit_qk_group(t, kind, n):
        """One projection psum group: q (kind=0) or k (kind=1) rows
        128t..128t+128 (heads 2t, 2t+1), l-chunk n. Lands directly in
        q_sb/k_sb (head 2t on partitions 0-63, head 2t+1 on 64-127)."""
        dst = (q_sb, k_sb)[kind][t]
        ocol = kind * HID + 128 * t
        ps = qkp.tile([128, 512], F32, tag="qkp", name=f"qk_ps_{kind}_{t}_{n}")
        for c in range(NT):
            nc.tensor.matmul(
                ps[:, :],
                lhsT=wq_sb[c][:, ocol : ocol + 128],
                rhs=x_sb[c][:, 512 * n : 512 * (n + 1)],
                start=(c == 0),
                stop=(c == NT - 1),
            )
        nc.vector.tensor_copy(dst[:, 512 * n : 512 * (n + 1)], ps[:, :])

    def emit_vt(jt):
        """V^T tile for key-block jt: [128 keys, 8 heads x (64 dims + ones)]."""
        ps = qkp.tile([128, 512], F32, tag="qkp", name=f"vt_ps_{jt}")
        for c in range(NT):
            nc.tensor.matmul(
                ps[:, :],
                lhsT=x_sb[c][:, 128 * jt : 128 * (jt + 1)],
                rhs=wq_sb[c][:, 2 * HID : 3 * HID],
                start=(c == 0),
                stop=(c == NT - 1),
            )
        vv = vt1[jt].rearrange("p (h e) -> p h e", e=65)
        nc.vector.tensor_copy(vv[:, :, 0:64], ps.rearrange("p (h d) -> p h d", d=64))
        nc.vector.memset(vv[:, :, 64:65], 1.0)

    def emit_pair(t, ic, interleave, vt_jit=False):
        """Attention for head pair (2t, 2t+1), i-chunk ic (512 queries).
        `interleave` closures emit independent PE work into the loop; with
        vt_jit the V^T tiles (from index 3) are emitted just-in-time ahead
        of the PV matmul that first needs them."""
        h0, h1 = 2 * t, 2 * t + 1
        ib = 512 * ic
        islice = slice(ib, ib + 512)
        ot0 = otp.tile([65, 512], F32, tag="ot0", name=f"ot0_{t}_{ic}")
        ot1 = otp.tile([65, 512], F32, tag="ot1", name=f"ot1_{t}_{ic}")

        def emit_st(jt):
            jslice = slice(128 * jt, 128 * (jt + 1))
            st = stp.tile([128, 1024], F32, tag="st", name=f"st_{t}_{ic}_{jt}")
            # the two K=64 matmuls run concurrently (PE row groups 0-1 / 2-3)
            nc.tensor.matmul(
                st[:, 0:512], lhsT=k_sb[t][0:64, jslice], rhs=q_sb[t][0:64, islice],
                start=True, stop=True,
            )
            nc.tensor.matmul(
                st[:, 512:1024], lhsT=k_sb[t][64:128, jslice],
                rhs=q_sb[t][64:128, islice],
                start=True, stop=True,
            )
            return st

        # software-pipelined emission: S^T for jt+1 is emitted BEFORE PV(jt)
        # and any interleave work, so in the PE's in-order stream the matmuls
        # feeding the next exp always run first and ScalarE never starves.
        slot = 0
        sts = {0: emit_st(0)}
        for jt in range(NJ):
            pt = ptp.tile([128, 1024], BF16, tag="pt", name=f"pt_{t}_{ic}_{jt}")
            st_t = sts.pop(jt)
            if jt in DVE_JTS:
                # VectorE exp: exp(16v) = (poly4(v))^16 - q weights are
                # host-prescaled by SCALE/16 so the matmul emits v directly
                p1 = scrp.tile([128, 1024], F32, tag="p1", name=f"p1_{t}_{ic}_{jt}")
                nc.vector._custom_dve(
                    EXP16_POLY, out=p1[:, :], in0=st_t[:, :],
                    s0=EXP_C[0], s1=EXP_C[1], imm2=EXP_C[2],
                )
                nc.vector._custom_dve(POW16, out=pt[:, :], in0=p1[:, :])
            else:
                nc.scalar.activation(pt[:, :], st_t[:, :], AF.Exp, scale=16.0)
            if jt + 1 < NJ:
                sts[jt + 1] = emit_st(jt + 1)
            if vt_jit and jt + 3 < NJ:
                emit_vt(jt + 3)
            vt = vt1[jt]
            nc.tensor.matmul(
                ot0[:, :], lhsT=vt[:, 65 * h0 : 65 * h0 + 65], rhs=pt[:, 0:512],
                start=(jt == 0), stop=(jt == NJ - 1),
            )
            nc.tensor.matmul(
                ot1[:, :], lhsT=vt[:, 65 * h1 : 65 * h1 + 65], rhs=pt[:, 512:1024],
                start=(jt == 0), stop=(jt == NJ - 1),
            )
            # fill PE slack with independent work, paced to finish by jt=13
            target = ((jt + 1) * len(interleave) + 13) // 14
            while slot < min(target, len(interleave)):
                interleave[slot]()
                slot += 1
        # softmax normalization: divide rows 0-63 by the ones-row (64).
        # The accumulator is evacuated in one copy so the psum tile frees
        # fast. reciprocal_approx_fast mis-reads non-zero partition offsets
        # on silicon, so the denominator row is staged to partition 0 first.
        for hh, ot in ((h0, ot0), (h1, ot1)):
            p = hh % 2
            o2u = smp.tile([65, 512], F32, tag=f"o2u{p}", name=f"o2u_{hh}_{ic}")
            nc.vector.tensor_copy(o2u[:, :], ot[:, :])
            den = smp.tile([1, 512], F32, tag=f"den{p}", name=f"den_{hh}_{ic}")
            nc.vector.tensor_copy(den[:, :], o2u[64:65, :])
            rec = smp.tile([1, 512], F32, tag=f"rec{p}", name=f"rec_{hh}_{ic}")
            nc.vector.reciprocal_approx_fast(rec[:, :], den[:, :])
            rb = smp.tile([64, 512], F32, tag=f"rb{p}", name=f"rb_{hh}_{ic}")
            nc.gpsimd.partition_broadcast(rb[:, :], rec[:, :])
            nc.vector.tensor_mul(o2[t][64 * p : 64 * p + 64, islice], o2u[0:64, :], rb[:, :])

    held_proj = {}

    def emit_proj_group(o, n, c_lo=0):
        if c_lo == 0:
            ps = qkp.tile([128, 512], F32, tag="qkp", name=f"y_ps_{o}_{n}")
        else:
            ps = held_proj.pop((o, n))
        for c in range(c_lo, NT):
            nc.tensor.matmul(
                ps[:, :],
                lhsT=wo_sb[c][:, 128 * o : 128 * (o + 1)],
                rhs=o2[c][:, 512 * n : 512 * (n + 1)],
                start=(c == 0),
                stop=(c == NT - 1),
            )
        yt = ytp.tile([128, 512], F32, tag="yt", name=f"yt_{o}_{n}")
        nc.vector.tensor_scalar_add(yt[:, :], ps[:, :], bias_sb[o][:, 0:1])
        nc.sync.dma_start(
            out_d[128 * o : 128 * (o + 1), 512 * n : 512 * (n + 1)], yt[:, :]
        )

    def emit_proj_partial(o, n):
        """First 3 channel-tiles of proj group (o, n); the psum tile is held
        and finished by emit_proj_group(o, n, c_lo=3) once the last pair's
        output is ready."""
        ps = qkp.tile([128, 512], F32, tag="qkp", name=f"y_ps_{o}_{n}")
        for c in range(3):
            nc.tensor.matmul(
                ps[:, :],
                lhsT=wo_sb[c][:, 128 * o : 128 * (o + 1)],
                rhs=o2[c][:, 512 * n : 512 * (n + 1)],
                start=(c == 0),
                stop=False,
            )
        held_proj[(o, n)] = ps

    # ---- emission schedule ----
    # pair 0's q (chunk 0) + full k projected up front; everything else is
    # interleaved just-in-time into earlier attention loops.
    emit_qk_group(0, 0, 0)
    for n in range(NL):
        emit_qk_group(0, 1, n)
    for jt in range(3):
        emit_vt(jt)

    # wo/bias loads off the critical startup path
    for c in range(NT):
        r = slice(128 * c, 128 * (c + 1))
        nc.sync.dma_start(wo_sb[c][:, :], woutT_d[r, :])
        nc.sync.dma_start(bias_sb[c][:, :], bias_d[r, :])

    def kg(t, n):
        return lambda: emit_qk_group(t, 1, n)

    def qg(t, n):
        return lambda: emit_qk_group(t, 0, n)

    def pj(o, n):
        return lambda: emit_proj_group(o, n)

    # pair t's q chunk for pass ic must be emitted BEFORE its (ic, t) loop
    # (the PE executes in order - a dependency later in its own stream would
    # deadlock). q chunks for pass ic+1 therefore fire during pass ic, and
    # proj chunk n fires during pass n+1.
    # later k chunks of a pair may fire early inside that pair's OWN loop
    # (k block n is first read at jt=4n, well after the interleave slot).
    inter = {
        (0, 0): [qg(1, 0), kg(1, 0)],  # vt tiles are emitted JIT (vt_jit)
        (0, 1): [kg(1, 1), kg(1, 2), kg(1, 3), qg(2, 0), kg(2, 0)],
        (0, 2): [kg(2, 1), kg(2, 2), kg(2, 3), qg(3, 0), kg(3, 0), qg(0, 1)],
        (0, 3): [kg(3, 1), kg(3, 2), kg(3, 3), qg(1, 1), qg(2, 1), qg(3, 1)],
        (1, 0): [pj(0, 0), qg(0, 2)],
        (1, 1): [pj(1, 0), qg(1, 2)],
        (1, 2): [pj(2, 0), qg(2, 2)],
        (1, 3): [pj(3, 0), qg(3, 2)],
        (2, 0): [pj(0, 1), qg(0, 3)],
        (2, 1): [pj(1, 1), qg(1, 3)],
        (2, 2): [pj(2, 1), qg(2, 3)],
        (2, 3): [pj(3, 1), qg(3, 3)],
        (3, 0): [pj(0, 2)],
        (3, 1): [pj(1, 2)],
        (3, 2): [pj(2, 2), pj(3, 2)],
        (3, 3): [
            lambda: emit_proj_partial(0, 3),
            lambda: emit_proj_partial(1, 3),
        ],
    }
    for ic in range(4):
        for t in range(NT):
            emit_pair(t, ic, inter.get((ic, t), []), vt_jit=(ic == 0 and t == 0))
    emit_proj_group(0, 3, c_lo=3)
    emit_proj_group(1, 3, c_lo=3)
    emit_proj_group(2, 3)
    emit_proj_group(3, 3)
    ctx.close()


_COMPILED = None


def _get_compiled():
    global _COMPILED
    if _COMPILED is None:
        nc = bacc.Bacc(
            "TRN2", target_bir_lowering=False, debug=False, num_devices=NCORES
        )
        x_d = nc.dram_tensor("x", [C, L], BF16, kind="ExternalInput").ap()
        wqkvT_d = nc.dram_tensor("wqkvT", [C, 3 * HID], BF16, kind="ExternalInput").ap()
        woutT_d = nc.dram_tensor("woutT", [HID, C], BF16, kind="ExternalInput").ap()
        bias_d = nc.dram_tensor("bias", [C, 1], F32, kind="ExternalInput").ap()
        out_d = nc.dram_tensor("out", [C, L], F32, kind="ExternalOutput").ap()
        with tile.TileContext(nc) as tc:
            build_kernel(tc, out_d, x_d, wqkvT_d, woutT_d, bias_d)
        nc.compile()
        _COMPILED = nc
    return _COMPILED


def make_in_maps(x, w_qkv, w_out, b_out):
    xb = np.asarray(x, dtype=np.float32).astype(ml_dtypes.bfloat16)
    wq_f = np.asarray(w_qkv, dtype=np.float32).T.copy()
    wq_f[:, 0:HID] *= SCALE / 16.0  # exp scale folded into the q projection
    wqkvT = np.ascontiguousarray(wq_f.astype(ml_dtypes.bfloat16))
    woutT = np.ascontiguousarray(
        np.asarray(w_out, dtype=np.float32).T.astype(ml_dtypes.bfloat16)
    )
    bias = np.ascontiguousarray(np.asarray(b_out, dtype=np.float32).reshape(C, 1))
    return [
        {
            "x": np.ascontiguousarray(xb[b]),
            "wqkvT": wqkvT,
            "woutT": woutT,
            "bias": bias,
        }
        for b in range(B)
    ]


LAST_RESULTS = None


def _install_ntff_hook():
    """Provide antenv.axon_hooks (absent from this image) so trace=True works."""
    import types

    try:
        from antenv.axon_hooks import get_axon_ntff_profile_hook  # noqa: F401

        return
    except ImportError:
        pass
    sys.path.insert(0, "/root/.axon_site")
    from trn_agent_boot.trn_boot import _ntff_profile_via_ctypes

    hook = _ntff_profile_via_ctypes("/opt/axon/libaxon_pjrt.so")
    import antenv

    mod = types.ModuleType("antenv.axon_hooks")
    mod._hook = hook
    mod.get_axon_ntff_profile_hook = lambda: mod._hook
    mod.set_axon_ntff_profile_hook = lambda h: setattr(mod, "_hook", h)
    sys.modules["antenv.axon_hooks"] = mod
    antenv.axon_hooks = mod
    # artifact upload has no egress in this container - make it a no-op
    bass_utils.upload_artifacts = lambda tmpdir: tmpdir


def kernel(x, w_qkv, w_out, b_out):
    global LAST_RESULTS
    nc = _get_compiled()
    in_maps = make_in_maps(x, w_qkv, w_out, b_out)
    trace = bool(int(os.environ.get("KERNEL_TRACE", "0")))
    if trace:
        _install_ntff_hook()
    res = bass_utils.run_bass_kernel_spmd(
        nc, in_maps, core_ids=list(range(NCORES)), trace=trace
    )
    LAST_RESULTS = res
    out = np.stack([np.asarray(res.results[b]["out"]) for b in range(B)])
    return out.astype(np.float32)


# revision 29
# speedup vs baseline: 1.0671x; 1.0671x over previous
"""Multi-head attention (B=8, C=512, L=2048, H=8, D=64) on 8 TRN2 NeuronCores.

Sharding: pure batch-parallel - core b computes batch b end-to-end (qkv proj,
8 heads of attention, out proj). No collectives.

Per-core layout strategy:
  - qkv projection with lhsT = w_qkv.T (host-transposed), rhs = x.
  - S^T = K^T Q  (keys on partitions) so the exp output is already the
    transposed P^T needed by the PV matmul, and no max-subtraction is needed
    (scores are ~N(0,1) after the 1/sqrt(D) scale, folded into exp's scale).
  - Heads are processed in pairs (2t, 2t+1) that live in partition halves
    0-63 / 64-127 of one qkv row-tile. The two K=64 S^T matmuls of a pair
    run CONCURRENTLY in the PE array (row groups 0-1 vs 2-3) and write the
    two 512-column halves of one [128, 1024] PSUM tile, so a single
    ScalarE exp instruction covers both heads.
  - PV uses lhsT = [V^T | ones] (65 columns): row 64 of the accumulator is
    the softmax denominator, computed for free.
  - V^T is computed directly from X (lhsT = X tiles), V is never materialized.
  - i is processed in 512-wide chunks (outer loop) so each chunk of the
    output projection overlaps the next chunk's attention pass.
"""

import os
import sys

sys.path.insert(0, "/opt/trn_rl_repo")

import numpy as np
import ml_dtypes

import concourse.bass as bass
import concourse.tile as tile
from concourse import bacc, mybir
from concourse import bass_utils

# ---- custom DVE exp: p = poly4(v), then p^16 (v = 0.125*S/16) -------------
# Offloads part of the softmax exp from the (bottleneck) ScalarE to VectorE.
from concourse.dve_spec import Spec, Src0, C0, C1, C2, One, sq, lower, _has_src1
import concourse.dve_ops as dve_ops
from concourse.dve_ops import DveOp
from concourse.dve_uop import DveOpSpec

EXP_C = (0.50053141, 0.16821747, 0.03882078)  # minimax on v in [-0.5125, 0.5125]


def _register_dve_op(name, spec):
    if name in dve_ops._SUB_OPCODE_FOR_NAME:
        return next(op for op in dve_ops.OPS if op.name == name)
    row = max(dve_ops._SUB_OPCODE_FOR_NAME.values()) + 1
    assert row < 0x20
    dve_ops._SUB_OPCODE_FOR_NAME[name] = row
    shas = {}
    for ver in ("v3", "v4"):
        s = DveOpSpec(
            name=name, opcode=row, uops=lower(spec, ver=ver), rd1_en=_has_src1(spec)
        )
        shas[ver] = s.sha(ver)
    op = DveOp(name, spec, subdim=False, uops_sha=shas)
    dve_ops.OPS.append(op)
    dve_ops.CUSTOM_DVE_SPECS[name] = spec
    return op


def _make_exp_ops():
    t = sq(Src0)
    spec1 = Spec(
        body=(One + Src0) + t * (C0 + C1 * Src0 + C2 * t),
        reference=lambda in0, in1, s0, s1, imm2: (
            1.0 + in0 + in0 * in0 * (s0 + s1 * in0 + imm2 * in0 * in0)
        ).astype(np.float32),
    )
    spec2 = Spec(
        body=sq(sq(sq(sq(Src0)))),
        reference=lambda in0, in1, s0, s1, imm2: (in0**16).astype(np.float32),
    )
    return (
        _register_dve_op("EXP16_POLY_ANT", spec1),
        _register_dve_op("POW16_ANT", spec2),
    )


EXP16_POLY, POW16 = _make_exp_ops()

# j-tiles (per 16-tile loop) whose exp runs on VectorE instead of ScalarE.
# Measured on HW: any offload loses (372-415us vs 358us) - the custom ops are
# fast (~1us) but holding an st PSUM slot on the in-order DVE queue stalls the
# S^T pipeline and re-throttles the PE clock. Default empty = all-ScalarE.
DVE_JTS = frozenset(
    int(x) for x in os.environ.get("KERNEL_DVE_JTS", "").split(",") if x != ""
)

B, C, L = 8, 512, 2048
H, D = 8, 64
HID = H * D  # 512
SCALE = float(D) ** -0.5
BF16 = mybir.dt.bfloat16
F32 = mybir.dt.float32
AF = mybir.ActivationFunctionType
NCORES = 8

NT = C // 128  # 4 channel tiles
NL = L // 512  # 4 l-chunks of 512
NJ = L // 128  # 16 key tiles


def build_kernel(tc, out_d, x_d, wqkvT_d, woutT_d, bias_d):
    nc = tc.nc
    from contextlib import ExitStack

    ctx = ExitStack()
    pers = ctx.enter_context(tc.tile_pool(name="pers", bufs=1))
    ptp = ctx.enter_context(tc.tile_pool(name="ptp", bufs=10))
    scrp = ctx.enter_context(tc.tile_pool(name="scrp", bufs=3))
    ytp = ctx.enter_context(tc.tile_pool(name="ytp", bufs=3))
    smp = ctx.enter_context(tc.tile_pool(name="smp", bufs=3))
    stp = ctx.enter_context(tc.tile_pool(name="stp", bufs=2, space="PSUM"))
    otp = ctx.enter_context(tc.tile_pool(name="otp", bufs=1, space="PSUM"))
    qkp = ctx.enter_context(tc.tile_pool(name="qkp", bufs=2, space="PSUM"))

    # ---- persistent SBUF tensors ----
    x_sb = [pers.tile([128, L], BF16, tag=f"x{c}", name=f"x{c}") for c in range(NT)]
    wq_sb = [
        pers.tile([128, 3 * HID], BF16, tag=f"wq{c}", name=f"wq{c}") for c in range(NT)
    ]
    wo_sb = [pers.tile([128, C], BF16, tag=f"wo{c}", name=f"wo{c}") for c in range(NT)]
    bias_sb = [
        pers.tile([128, 1], F32, tag=f"bias{c}", name=f"bias{c}") for c in range(NT)
    ]
    q_sb = [pers.tile([128, L], BF16, tag=f"q{t}", name=f"q{t}") for t in range(NT)]
    k_sb = [pers.tile([128, L], BF16, tag=f"k{t}", name=f"k{t}") for t in range(NT)]
    vt1 = [
        pers.tile([128, H * 65], BF16, tag=f"vt{j}", name=f"vt{j}") for j in range(NJ)
    ]
    o2 = [pers.tile([128, L], BF16, tag=f"o2_{c}", name=f"o2_{c}") for c in range(NT)]

    # ---- input DMAs (x first - it gates the qkv projection; halves engage
    # more DMA queues in parallel; wo/bias load later, off the hot path) ----
    for c in range(NT):
        r = slice(128 * c, 128 * (c + 1))
        nc.sync.dma_start(x_sb[c][:, 0:1024], x_d[r, 0:1024])
        nc.sync.dma_start(x_sb[c][:, 1024:2048], x_d[r, 1024:2048])
        nc.sync.dma_start(wq_sb[c][:, 0:1024], wqkvT_d[r, 0:1024])
    for c in range(NT):
        r = slice(128 * c, 128 * (c + 1))
        nc.sync.dma_start(wq_sb[c][:, 1024:1536], wqkvT_d[r, 1024:1536])

    # ---- PE warm-up: dummy matmuls during the input-DMA window so the HAM
    # clock gate opens (1.2 -> 2.4 GHz) before the real work arrives. The
    # chain ends in a DMA to an internal DRAM scratch so DCE keeps it. ----
    warm_scratch = nc.dram_tensor("warm_scratch", [128, 512], F32)
    warm_sb = pers.tile([128, 512], BF16, tag="warm", name="warm_sb")
    warm_out = pers.tile([128, 512], F32, tag="warmo", name="warm_out")
    nc.vector.memset(warm_sb[:, :], 0.001)
    wps = qkp.tile([128, 512], F32, tag="qkp", name="warm_ps")
    for w in range(40):
        nc.tensor.matmul(
            wps[:, :], lhsT=warm_sb[:, 0:128], rhs=warm_sb[:, :],
            start=True, stop=True,
        )
    nc.vector.tensor_copy(warm_out[:, :], wps[:, :])
    nc.sync.dma_start(warm_scratch.ap()[:, :], warm_out[:, :])

    def emit_qk_group(t, kind, n):
        """One projection psum group: q (kind=0) or k (kind=1) rows
        128t..128t+128 (heads 2t, 2t+1), l-chunk n. Lands directly in
        q_sb/k_sb (head 2t on partitions 0-63, head 2t+1 on 64-127)."""
        dst = (q_sb, k_sb)[kind][t]
        ocol = kind * HID + 128 * t
        ps = qkp.tile([128, 512], F32, tag="qkp", name=f"qk_ps_{kind}_{t}_{n}")
        for c in range(NT):
            nc.tensor.matmul(
                ps[:, :],
                lhsT=wq_sb[c][:, ocol : ocol + 128],
                rhs=x_sb[c][:, 512 * n : 512 * (n + 1)],
                start=(c == 0),
                stop=(c == NT - 1),
            )
        nc.vector.tensor_copy(dst[:, 512 * n : 512 * (n + 1)], ps[:, :])

    def emit_vt(jt):
        """V^T tile for key-block jt: [128 keys, 8 heads x (64 dims + ones)]."""
        ps = qkp.tile([128, 512], F32, tag="qkp", name=f"vt_ps_{jt}")
        for c in range(NT):
            nc.tensor.matmul(
                ps[:, :],
                lhsT=x_sb[c][:, 128 * jt : 128 * (jt + 1)],
                rhs=wq_sb[c][:, 2 * HID : 3 * HID],
                start=(c == 0),
                stop=(c == NT - 1),
            )
        vv = vt1[jt].rearrange("p (h e) -> p h e", e=65)
        nc.vector.tensor_copy(vv[:, :, 0:64], ps.rearrange("p (h d) -> p h d", d=64))
        nc.vector.memset(vv[:, :, 64:65], 1.0)

    def emit_pair(t, ic, interleave, vt_jit=False):
        """Attention for head pair (2t, 2t+1), i-chunk ic (512 queries).
        `interleave` closures emit independent PE work into the loop; with
        vt_jit the V^T tiles (from index 3) are emitted just-in-time ahead
        of the PV matmul that first needs them."""
        h0, h1 = 2 * t, 2 * t + 1
        ib = 512 * ic
        islice = slice(ib, ib + 512)
        ot0 = otp.tile([65, 512], F32, tag="ot0", name=f"ot0_{t}_{ic}")
        ot1 = otp.tile([65, 512], F32, tag="ot1", name=f"ot1_{t}_{ic}")

        def emit_st(jt):
            jslice = slice(128 * jt, 128 * (jt + 1))
            st = stp.tile([128, 1024], F32, tag="st", name=f"st_{t}_{ic}_{jt}")
            # the two K=64 matmuls run concurrently (PE row groups 0-1 / 2-3)
            nc.tensor.matmul(
                st[:, 0:512], lhsT=k_sb[t][0:64, jslice], rhs=q_sb[t][0:64, islice],
                start=True, stop=True,
            )
            nc.tensor.matmul(
                st[:, 512:1024], lhsT=k_sb[t][64:128, jslice],
                rhs=q_sb[t][64:128, islice],
                start=True, stop=True,
            )
            return st

        # software-pipelined emission: S^T for jt+1 is emitted BEFORE PV(jt)
        # and any interleave work, so in the PE's in-order stream the matmuls
        # feeding the next exp always run first and ScalarE never starves.
        slot = 0
        sts = {0: emit_st(0)}
        for jt in range(NJ):
            pt = ptp.tile([128, 1024], BF16, tag="pt", name=f"pt_{t}_{ic}_{jt}")
            st_t = sts.pop(jt)
            if jt in DVE_JTS:
                # VectorE exp: exp(16v) = (poly4(v))^16 - q weights are
                # host-prescaled by SCALE/16 so the matmul emits v directly
                p1 = scrp.tile([128, 1024], F32, tag="p1", name=f"p1_{t}_{ic}_{jt}")
                nc.vector._custom_dve(
                    EXP16_POLY, out=p1[:, :], in0=st_t[:, :],
                    s0=EXP_C[0], s1=EXP_C[1], imm2=EXP_C[2],
                )
                nc.vector._custom_dve(POW16, out=pt[:, :], in0=p1[:, :])
            else:
                nc.scalar.activation(pt[:, :], st_t[:, :], AF.Exp, scale=16.0)
            if jt + 1 < NJ:
                sts[jt + 1] = emit_st(jt + 1)
            if vt_jit and jt + 3 < NJ:
                emit_vt(jt + 3)
            vt = vt1[jt]
            nc.tensor.matmul(
                ot0[:, :], lhsT=vt[:, 65 * h0 : 65 * h0 + 65], rhs=pt[:, 0:512],
                start=(jt == 0), stop=(jt == NJ - 1),
            )
            nc.tensor.matmul(
                ot1[:, :], lhsT=vt[:, 65 * h1 : 65 * h1 + 65], rhs=pt[:, 512:1024],
                start=(jt == 0), stop=(jt == NJ - 1),
            )
            # fill PE slack with independent work, paced to finish by jt=13
            target = ((jt + 1) * len(interleave) + 13) // 14
            while slot < min(target, len(interleave)):
                interleave[slot]()
                slot += 1
        # softmax normalization: divide rows 0-63 by the ones-row (64).
        # The accumulator is evacuated in one copy so the psum tile frees
        # fast. reciprocal_approx_fast mis-reads non-zero partition offsets
        # on silicon, so the denominator row is staged to partition 0 first.
        for hh, ot in ((h0, ot0), (h1, ot1)):
            p = hh % 2
            o2u = smp.tile([65, 512], F32, tag=f"o2u{p}", name=f"o2u_{hh}_{ic}")
            nc.vector.tensor_copy(o2u[:, :], ot[:, :])
            den = smp.tile([1, 512], F32, tag=f"den{p}", name=f"den_{hh}_{ic}")
            nc.vector.tensor_copy(den[:, :], o2u[64:65, :])
            rec = smp.tile([1, 512], F32, tag=f"rec{p}", name=f"rec_{hh}_{ic}")
            nc.vector.reciprocal_approx_fast(rec[:, :], den[:, :])
            rb = smp.tile([64, 512], F32, tag=f"rb{p}", name=f"rb_{hh}_{ic}")
            nc.gpsimd.partition_broadcast(rb[:, :], rec[:, :])
            nc.vector.tensor_mul(o2[t][64 * p : 64 * p + 64, islice], o2u[0:64, :], rb[:, :])

    held_proj = {}

    def emit_proj_group(o, n, c_lo=0):
        if c_lo == 0:
            ps = qkp.tile([128, 512], F32, tag="qkp", name=f"y_ps_{o}_{n}")
        else:
            ps = held_proj.pop((o, n))
        for c in range(c_lo, NT):
            nc.tensor.matmul(
                ps[:, :],
                lhsT=wo_sb[c][:, 128 * o : 128 * (o + 1)],
                rhs=o2[c][:, 512 * n : 512 * (n + 1)],
                start=(c == 0),
                stop=(c == NT - 1),
            )
        yt = ytp.tile([128, 512], F32, tag="yt", name=f"yt_{o}_{n}")
        nc.vector.tensor_scalar_add(yt[:, :], ps[:, :], bias_sb[o][:, 0:1])
        nc.sync.dma_start(
            out_d[128 * o : 128 * (o + 1), 512 * n : 512 * (n + 1)], yt[:, :]
        )

    def emit_proj_partial(o, n):
        """First 3 channel-tiles of proj group (o, n); the psum tile is held
        and finished by emit_proj_group(o, n, c_lo=3) once the last pair's
        output is ready."""
        ps = qkp.tile([128, 512], F32, tag="qkp", name=f"y_ps_{o}_{n}")
        for c in range(3):
            nc.tensor.matmul(
                ps[:, :],
                lhsT=wo_sb[c][:, 128 * o : 128 * (o + 1)],
                rhs=o2[c][:, 512 * n : 512 * (n + 1)],
                start=(c == 0),
                stop=False,
            )
        held_proj[(o, n)] = ps

    # ---- emission schedule ----
    # pair 0's q (chunk 0) + full k projected up front; everything else is
    # interleaved just-in-time into earlier attention loops.
    emit_qk_group(0, 0, 0)
    for n in range(NL):
        emit_qk_group(0, 1, n)
    for jt in range(3):
        emit_vt(jt)

    # wo/bias loads off the critical startup path
    for c in range(NT):
        r = slice(128 * c, 128 * (c + 1))
        nc.sync.dma_start(wo_sb[c][:, :], woutT_d[r, :])
        nc.sync.dma_start(bias_sb[c][:, :], bias_d[r, :])

    def kg(t, n):
        return lambda: emit_qk_group(t, 1, n)

    def qg(t, n):
        return lambda: emit_qk_group(t, 0, n)

    def pj(o, n):
        return lambda: emit_proj_group(o, n)

    # pair t's q chunk for pass ic must be emitted BEFORE its (ic, t) loop
    # (the PE executes in order - a dependency later in its own stream would
    # deadlock). q chunks for pass ic+1 therefore fire during pass ic, and
    # proj chunk n fires during pass n+1.
    # later k chunks of a pair may fire early inside that pair's OWN loop
    # (k block n is first read at jt=4n, well after the interleave slot).
    inter = {
        (0, 0): [qg(1, 0), kg(1, 0)],  # vt tiles are emitted JIT (vt_jit)
        (0, 1): [kg(1, 1), kg(1, 2), kg(1, 3), qg(2, 0), kg(2, 0)],
        (0, 2): [kg(2, 1), kg(2, 2), kg(2, 3), qg(3, 0), kg(3, 0), qg(0, 1)],
        (0, 3): [kg(3, 1), kg(3, 2), kg(3, 3), qg(1, 1), qg(2, 1), qg(3, 1)],
        (1, 0): [pj(0, 0), qg(0, 2)],
        (1, 1): [pj(1, 0), qg(1, 2)],
        (1, 2): [pj(2, 0), qg(2, 2)],
        (1, 3): [pj(3, 0), qg(3, 2)],
        (2, 0): [pj(0, 1), qg(0, 3)],
        (2, 1): [pj(1, 1), qg(1, 3)],
        (2, 2): [pj(2, 1), qg(2, 3)],
        (2, 3): [pj(3, 1), qg(3, 3)],
        (3, 0): [pj(0, 2)],
        (3, 1): [pj(1, 2)],
        (3, 2): [pj(2, 2), pj(3, 2)],
        (3, 3): [
            lambda: emit_proj_partial(0, 3),
            lambda: emit_proj_partial(1, 3),
        ],
    }
    for ic in range(4):
        for t in range(NT):
            emit_pair(t, ic, inter.get((ic, t), []), vt_jit=(ic == 0 and t == 0))
    emit_proj_group(0, 3, c_lo=3)
    emit_proj_group(1, 3, c_lo=3)
    emit_proj_group(2, 3)
    emit_proj_group(3, 3)
    ctx.close()


_COMPILED = None


def _get_compiled():
    global _COMPILED
    if _COMPILED is None:
        nc = bacc.Bacc(
            "TRN2", target_bir_lowering=False, debug=False, num_devices=NCORES
        )
        x_d = nc.dram_tensor("x", [C, L], BF16, kind="ExternalInput").ap()
        wqkvT_d = nc.dram_tensor("wqkvT", [C, 3 * HID], BF16, kind="ExternalInput").ap()
        woutT_d = nc.dram_tensor("woutT", [HID, C], BF16, kind="ExternalInput").ap()
        bias_d = nc.dram_tensor("bias", [C, 1], F32, kind="ExternalInput").ap()
        out_d = nc.dram_tensor("out", [C, L], F32, kind="ExternalOutput").ap()
        with tile.TileContext(nc) as tc:
            build_kernel(tc, out_d, x_d, wqkvT_d, woutT_d, bias_d)
        nc.compile()
        _COMPILED = nc
    return _COMPILED


def make_in_maps(x, w_qkv, w_out, b_out):
    xb = np.asarray(x, dtype=np.float32).astype(ml_dtypes.bfloat16)
    wq_f = np.asarray(w_qkv, dtype=np.float32).T.copy()
    wq_f[:, 0:HID] *= SCALE / 16.0  # exp scale folded into the q projection
    wqkvT = np.ascontiguousarray(wq_f.astype(ml_dtypes.bfloat16))
    woutT = np.ascontiguousarray(
        np.asarray(w_out, dtype=np.float32).T.astype(ml_dtypes.bfloat16)
    )
    bias = np.ascontiguousarray(np.asarray(b_out, dtype=np.float32).reshape(C, 1))
    return [
        {
            "x": np.ascontiguousarray(xb[b]),
            "wqkvT": wqkvT,
            "woutT": woutT,
            "bias": bias,
        }
        for b in range(B)
    ]


LAST_RESULTS = None


def _install_ntff_hook():
    """Provide antenv.axon_hooks (absent from this image) so trace=True works."""
    import types

    try:
        from antenv.axon_hooks import get_axon_ntff_profile_hook  # noqa: F401

        return
    except ImportError:
        pass
    sys.path.insert(0, "/root/.axon_site")
    from trn_agent_boot.trn_boot import _ntff_profile_via_ctypes

    hook = _ntff_profile_via_ctypes("/opt/axon/libaxon_pjrt.so")
    import antenv

    mod = types.ModuleType("antenv.axon_hooks")
    mod._hook = hook
    mod.get_axon_ntff_profile_hook = lambda: mod._hook
    mod.set_axon_ntff_profile_hook = lambda h: setattr(mod, "_hook", h)
    sys.modules["antenv.axon_hooks"] = mod
    antenv.axon_hooks = mod
    # artifact upload has no egress in this container - make it a no-op
    bass_utils.upload_artifacts = lambda tmpdir: tmpdir


def kernel(x, w_qkv, w_out, b_out):
    global LAST_RESULTS
    nc = _get_compiled()
    in_maps = make_in_maps(x, w_qkv, w_out, b_out)
    trace = bool(int(os.environ.get("KERNEL_TRACE", "0")))
    if trace:
        _install_ntff_hook()
    res = bass_utils.run_bass_kernel_spmd(
        nc, in_maps, core_ids=list(range(NCORES)), trace=trace
    )
    LAST_RESULTS = res
    out = np.stack([np.asarray(res.results[b]["out"]) for b in range(B)])
    return out.astype(np.float32)
